# revision 1
# baseline (speedup 1.0000x reference)
"""Trainium2 Bass kernel for a LoRA-MoE layer (gate top-2 softmax routing +
dense base linear + per-expert low-rank adapters), SPMD across 8 NeuronCores.

Math (per token t):
    logits = x @ gate_w.T                      # [E]
    top-2 softmax over logits -> dense w[E] (0 for non-selected)
    out = x @ base_w.T + base_b
        + SCALING * sum_e w[e] * (x @ lora_A[e].T) @ lora_B[e].T

Key identity used: with w folded into the rank-space activations,
    lora_out = (low * w_rep) @ B_all.T,  low = x @ A_all.T   (A_all: [E*R, D])
so the whole MoE-LoRA is two dense matmuls + tiny gating vector math.

Sharding: 4-way over tokens x 2-way over out-features (8 cores, no
collectives).  Per core: T=1024 tokens, TO=2048 out features.

Layout per core (everything "transposed", contraction dim on partitions):
    out.T[o, t] = sum_d W[o, d] * x.T[d, t]    (x.T moving, W tiles stationary)
"""

import numpy as np

import concourse.bass as bass
import concourse.bass_isa as bass_isa
import concourse.mybir as mybir
import concourse.tile as tile
from concourse import bacc
from concourse.bass_utils import run_bass_kernel_spmd

F32 = mybir.dt.float32
F32R = mybir.dt.float32r

# Problem constants
B, S, D, O = 2, 2048, 4096, 4096
E, R = 8, 16
ER = E * R  # 128
SCALING = 32.0 / 16.0

# Sharding: 4 token groups x 2 out-feature groups
N_CORES = 8
TG, OG = 4, 2
T = (B * S) // TG       # 1024 tokens per core
TO = O // OG            # 2048 out features per core
KT = D // 128           # 32 contraction tiles
OTN = TO // 128         # 16 out tiles per core
CH = 2                  # token chunks of 512
CHW = T // CH           # 512


def build_body(nc, tc, tensors, mm_dt=F32R, gate_dt=F32R):
    xT, wT, aT, gT, bT, bias2, Rm, out = tensors
    AX_C = mybir.AxisListType.C
    OP = mybir.AluOpType

    with (
        tc.tile_pool(name="xp", bufs=KT) as xp,
        tc.tile_pool(name="wp", bufs=4) as wp,
        tc.tile_pool(name="cst", bufs=1) as cst,
        tc.tile_pool(name="apl", bufs=3) as apl,
        tc.tile_pool(name="gw", bufs=1) as gw,
        tc.tile_pool(name="outp", bufs=2) as outp,
        tc.tile_pool(name="psA", bufs=2, space="PSUM") as psA,
        tc.tile_pool(name="psB", bufs=4, space="PSUM") as psB,
    ):
        # ---- constants ----
        bT_sb = cst.tile([ER, TO], F32R)
        nc.gpsimd.dma_start(out=bT_sb[:], in_=bT[:].bitcast(F32R))
        bias_sb = cst.tile([128, OTN], F32)
        nc.gpsimd.dma_start(out=bias_sb[:], in_=bias2[:])
        Rm_sb = cst.tile([E, ER], F32R)
        nc.gpsimd.dma_start(out=Rm_sb[:], in_=Rm[:].bitcast(F32R))

        # ---- resident x.T tiles ----
        x_tiles = []
        for k in range(KT):
            xk = xp.tile([128, T], F32R, tag="x", name=f"x{k}")
            nc.sync.dma_start(out=xk[:], in_=xT[:, k, :].bitcast(F32R))
            x_tiles.append(xk)

        # ---- phase A: low.T = A_all.T^T @ x.T ; gate.T = g^T @ x.T ----
        low_ps = [psA.tile([ER, CHW], F32, tag="low", name=f"lowps{c}") for c in range(CH)]
        gate_ps = [psA.tile([E, CHW], F32, tag="gate", name=f"gateps{c}") for c in range(CH)]
        for k in range(KT):
            ak = apl.tile([128, ER], F32R, tag="a", name=f"a{k}")
            nc.gpsimd.dma_start(out=ak[:], in_=aT[:, k, :].bitcast(F32R))
            gk = apl.tile([128, E], F32R, tag="g", name=f"g{k}")
            nc.gpsimd.dma_start(out=gk[:], in_=gT[:, k, :].bitcast(F32R))
            for c in range(CH):
                rhs = x_tiles[k][:, c * CHW:(c + 1) * CHW]
                nc.tensor.matmul(low_ps[c][:], lhsT=ak[:], rhs=rhs,
                                 start=(k == 0), stop=(k == KT - 1))
                nc.tensor.matmul(gate_ps[c][:], lhsT=gk[:], rhs=rhs,
                                 start=(k == 0), stop=(k == KT - 1))

        # ---- gating math in [E, t] layout, per 512-token chunk ----
        lowT_sb = gw.tile([ER, T], F32R, tag="lowT")
        for c in range(CH):
            cs = slice(c * CHW, (c + 1) * CHW)
            g_sb = gw.tile([E, CHW], F32, tag="gsb", name=f"gsb{c}")
            nc.vector.tensor_copy(g_sb[:], gate_ps[c][:])
            m1b = gw.tile([E, CHW], F32, tag="m1b", name=f"m1b{c}")
            nc.gpsimd.partition_all_reduce(m1b[:], g_sb[:], channels=E,
                                           reduce_op=bass_isa.ReduceOp.max)
            eq = gw.tile([E, CHW], F32, tag="tmp", bufs=3, name=f"eq{c}")
            nc.vector.tensor_tensor(eq[:], g_sb[:], m1b[:], op=OP.is_equal)
            gm = gw.tile([E, CHW], F32, tag="tmp", bufs=3, name=f"gm{c}")
            nc.vector.scalar_tensor_tensor(gm[:], in0=eq[:], scalar=-1e30, in1=g_sb[:],
                                           op0=OP.mult, op1=OP.add)
            m2b = gw.tile([E, CHW], F32, tag="m2b", name=f"m2b{c}")
            nc.gpsimd.partition_all_reduce(m2b[:], gm[:], channels=E,
                                           reduce_op=bass_isa.ReduceOp.max)
            diff = gw.tile([E, CHW], F32, tag="tmp", bufs=3, name=f"diff{c}")
            nc.vector.tensor_sub(diff[:], g_sb[:], m1b[:])
            ex = gw.tile([E, CHW], F32, tag="ex", name=f"ex{c}")
            nc.scalar.activation(ex[:], diff[:], mybir.ActivationFunctionType.Exp)
            mask = gw.tile([E, CHW], F32, tag="tmp", bufs=3, name=f"mask{c}")
            nc.vector.tensor_tensor(mask[:], g_sb[:], m2b[:], op=OP.is_ge)
            wn = gw.tile([E, CHW], F32, tag="wn", name=f"wn{c}")
            nc.vector.tensor_mul(wn[:], ex[:], mask[:])
            # denominator 1 + exp(m2 - m1), computed broadcast on all 8 rows
            dmb = gw.tile([E, CHW], F32, tag="tmp", bufs=3, name=f"dmb{c}")
            nc.vector.tensor_sub(dmb[:], m2b[:], m1b[:])
            edb = gw.tile([E, CHW], F32, tag="edb", name=f"edb{c}")
            nc.scalar.activation(edb[:], dmb[:], mybir.ActivationFunctionType.Exp)
            denb = gw.tile([E, CHW], F32, tag="tmp", bufs=3, name=f"denb{c}")
            nc.vector.tensor_scalar_add(denb[:], edb[:], 1.0)
            recb = gw.tile([E, CHW], F32, tag="recb", name=f"recb{c}")
            nc.vector.reciprocal(recb[:], denb[:])
            wsc = gw.tile([E, CHW], F32R, tag="wsc", name=f"wsc{c}")
            nc.vector.scalar_tensor_tensor(wsc[:], in0=wn[:], scalar=SCALING, in1=recb[:],
                                           op0=OP.mult, op1=OP.mult)
            # replicate each expert weight over its 16 ranks via tiny matmul
            wrep_ps = psA.tile([ER, CHW], F32, tag="gate", name=f"wrep{c}")
            nc.tensor.matmul(wrep_ps[:], lhsT=Rm_sb[:], rhs=wsc[:],
                             start=True, stop=True)
            # low_w.T = low.T * w_rep  (copy wrep to SBUF first: DVE has a
            # single PSUM read port, two-PSUM-operand tensor_tensor is illegal)
            wrep_sb = gw.tile([ER, CHW], F32, tag="wrepsb", name=f"wrepsb{c}")
            nc.scalar.copy(wrep_sb[:], wrep_ps[:])
            nc.vector.tensor_tensor(lowT_sb[:, cs], low_ps[c][:], wrep_sb[:], op=OP.mult)

        # ---- phase B: out.T tiles = W-tile^T @ x.T  (+ B-tile^T @ low_w.T) ----
        for ot in range(OTN):
            wtiles = []
            for q in range(4):
                wq = wp.tile([128, 8, 128], F32R, tag="w", name=f"w{ot}_{q}")
                nc.scalar.dma_start(out=wq[:], in_=wT[:, ot, q * 8:(q + 1) * 8, :].bitcast(F32R))
                wtiles.append(wq)
            pb = [psB.tile([128, CHW], F32, tag="pb", name=f"pb{ot}_{c}") for c in range(CH)]
            for k in range(KT):
                wk = wtiles[k // 8][:, k % 8, :]
                for c in range(CH):
                    nc.tensor.matmul(pb[c][:], lhsT=wk,
                                     rhs=x_tiles[k][:, c * CHW:(c + 1) * CHW],
                                     start=(k == 0), stop=False)
            for c in range(CH):
                nc.tensor.matmul(pb[c][:], lhsT=bT_sb[:, ot * 128:(ot + 1) * 128],
                                 rhs=lowT_sb[:, c * CHW:(c + 1) * CHW],
                                 start=False, stop=True)
            o_sb = outp.tile([128, T], F32, tag="o", name=f"o{ot}")
            for c in range(CH):
                nc.vector.tensor_scalar(o_sb[:, c * CHW:(c + 1) * CHW], pb[c][:],
                                        scalar1=bias_sb[:, ot:ot + 1], scalar2=None,
                                        op0=OP.add)
            nc.gpsimd.dma_start(out=out[:, ot, :], in_=o_sb[:])


def build_module(mm_dt=F32R, gate_dt=F32R, debug=False):
    nc = bacc.Bacc("TRN2", target_bir_lowering=False, debug=debug)
    xT = nc.dram_tensor("xT", [128, KT, T], F32, kind="ExternalInput")
    wT = nc.dram_tensor("wT", [128, OTN, KT, 128], F32, kind="ExternalInput")
    aT = nc.dram_tensor("aT", [128, KT, ER], F32, kind="ExternalInput")
    gT = nc.dram_tensor("gT", [128, KT, E], F32, kind="ExternalInput")
    bT = nc.dram_tensor("bT", [ER, TO], F32, kind="ExternalInput")
    bias2 = nc.dram_tensor("bias2", [128, OTN], F32, kind="ExternalInput")
    Rm = nc.dram_tensor("Rm", [E, ER], F32, kind="ExternalInput")
    out = nc.dram_tensor("out", [128, OTN, T], F32, kind="ExternalOutput")
    with tile.TileContext(nc) as tc:
        build_body(nc, tc, (xT, wT, aT, gT, bT, bias2, Rm, out),
                   mm_dt=mm_dt, gate_dt=gate_dt)
    nc.compile()
    return nc


def shard_inputs(x, gate_w, base_w, base_b, lora_A, lora_B):
    """FULL inputs -> list of 8 per-core input maps (host-side, free)."""
    x = np.asarray(x, dtype=np.float32)
    gate_w = np.asarray(gate_w, dtype=np.float32)
    base_w = np.asarray(base_w, dtype=np.float32)
    base_b = np.asarray(base_b, dtype=np.float32)
    lora_A = np.asarray(lora_A, dtype=np.float32)
    lora_B = np.asarray(lora_B, dtype=np.float32)

    xf = x.reshape(B * S, D)
    # replicated smalls
    gT = np.ascontiguousarray(gate_w.T.reshape(KT, 128, E).transpose(1, 0, 2))
    A_flat = lora_A.reshape(ER, D)
    aT = np.ascontiguousarray(A_flat.T.reshape(KT, 128, ER).transpose(1, 0, 2))
    B_flat = lora_B.transpose(0, 2, 1).reshape(ER, O)   # [er, o]
    Rm = np.repeat(np.eye(E, dtype=np.float32), R, axis=1)  # [E, ER]

    in_maps = []
    for c in range(N_CORES):
        tg, og = c // OG, c % OG
        x_c = xf[tg * T:(tg + 1) * T]                       # [T, D]
        xT = np.ascontiguousarray(x_c.T.reshape(KT, 128, T).transpose(1, 0, 2))
        w_c = base_w[og * TO:(og + 1) * TO]                 # [TO, D]
        wT = np.ascontiguousarray(
            w_c.reshape(OTN, 128, KT, 128).transpose(3, 0, 2, 1))
        bT = np.ascontiguousarray(B_flat[:, og * TO:(og + 1) * TO])
        bias2 = np.ascontiguousarray(base_b[og * TO:(og + 1) * TO].reshape(OTN, 128).T)
        in_maps.append({"xT": xT, "wT": wT, "aT": aT, "gT": gT,
                        "bT": bT, "bias2": bias2, "Rm": Rm})
    return in_maps


def gather_outputs(results):
    """list of 8 per-core result maps -> FULL output [B, S, O]."""
    full = np.empty((B * S, O), dtype=np.float32)
    for c in range(N_CORES):
        tg, og = c // OG, c % OG
        oc = results[c]["out"]                              # [128, OTN, T]
        full[tg * T:(tg + 1) * T, og * TO:(og + 1) * TO] = \
            oc.transpose(2, 1, 0).reshape(T, TO)
    return full.reshape(B, S, O)


_NC_CACHE = {}


def _get_module(mm_dt=F32R, gate_dt=F32R):
    key = (mm_dt, gate_dt)
    if key not in _NC_CACHE:
        _NC_CACHE[key] = build_module(mm_dt=mm_dt, gate_dt=gate_dt)
    return _NC_CACHE[key]


def run_sharded(in_maps, mm_dt=F32R, gate_dt=F32R, **run_kwargs):
    nc = _get_module(mm_dt=mm_dt, gate_dt=gate_dt)
    return run_bass_kernel_spmd(nc, in_maps, list(range(N_CORES)), **run_kwargs)


def kernel(x, gate_w, base_w, base_b, lora_A, lora_B):
    in_maps = shard_inputs(x, gate_w, base_w, base_b, lora_A, lora_B)
    res = run_sharded(in_maps)
    return gather_outputs(res.results)



# revision 2
# speedup vs baseline: 1.2655x; 1.2655x over previous
"""Trainium2 Bass kernel for a LoRA-MoE layer (gate top-2 softmax routing +
dense base linear + per-expert low-rank adapters), SPMD across 8 NeuronCores.

Math (per token t):
    logits = x @ gate_w.T                      # [E]
    top-2 softmax over logits -> dense w[E] (0 for non-selected)
    out = x @ base_w.T + base_b
        + SCALING * sum_e w[e] * (x @ lora_A[e].T) @ lora_B[e].T

Key identity used: with w folded into the rank-space activations,
    lora_out = (low * w_rep) @ B_all.T,  low = x @ A_all.T   (A_all: [E*R, D])
so the whole MoE-LoRA is two dense matmuls + tiny gating vector math.

Sharding: 8-way over tokens (512 tokens/core, full 4096 out features).
Token-only sharding means the LoRA-A + gate matmuls are not replicated
across out-feature groups, halving that tensor-engine work vs a 4x2 split.

All matmul operands are bf16 (same 1 cycle/row PE rate as f32r, same
effective precision, half the HBM traffic); PSUM accumulation is fp32.

Layout per core (everything "transposed", contraction dim on partitions):
    out.T[o, t] = sum_d W[o, d] * x.T[d, t]    (x.T moving, W tiles stationary)
"""

import numpy as np
import ml_dtypes

import concourse.bass as bass
import concourse.bass_isa as bass_isa
import concourse.mybir as mybir
import concourse.tile as tile
from concourse import bacc
from concourse.bass_utils import run_bass_kernel_spmd

F32 = mybir.dt.float32
BF16 = mybir.dt.bfloat16
NP_BF16 = ml_dtypes.bfloat16

# Problem constants
B, S, D, O = 2, 2048, 4096, 4096
E, R = 8, 16
ER = E * R  # 128
SCALING = 32.0 / 16.0

# Sharding: 8 token groups, full out-feature range per core
N_CORES = 8
TG = 8
T = (B * S) // TG       # 512 tokens per core
TO = O                  # 4096 out features per core
KT = D // 128           # 32 contraction tiles
OTN = TO // 128         # 32 out tiles per core
XC = 8                  # x DMA chunk: 8 k-tiles (1 MiB bf16)


def build_body(nc, tc, tensors):
    xT, wT, aT, gT, bT, bias2, Rm, out = tensors
    OP = mybir.AluOpType

    with (
        tc.tile_pool(name="xp", bufs=KT // XC) as xp,
        tc.tile_pool(name="wp", bufs=4) as wp,
        tc.tile_pool(name="cst", bufs=1) as cst,
        tc.tile_pool(name="gw", bufs=1) as gw,
        tc.tile_pool(name="outp", bufs=3) as outp,
        tc.tile_pool(name="psA", bufs=1, space="PSUM") as psA,
        tc.tile_pool(name="psG", bufs=2, space="PSUM") as psG,
        tc.tile_pool(name="psB", bufs=5, space="PSUM") as psB,
    ):
        # ---- small constants (gpsimd queue; done long before needed) ----
        a_sb = cst.tile([128, KT, ER], BF16)
        nc.gpsimd.dma_start(out=a_sb[:], in_=aT[:])
        g_sb = cst.tile([128, KT, E], BF16)
        nc.gpsimd.dma_start(out=g_sb[:], in_=gT[:])
        Rm_sb = cst.tile([E, ER], BF16)
        nc.gpsimd.dma_start(out=Rm_sb[:], in_=Rm[:])
        bias_sb = cst.tile([128, OTN], F32)
        nc.gpsimd.dma_start(out=bias_sb[:], in_=bias2[:])

        # ---- resident x.T tiles: 4 chunked DMAs on the sync queue ----
        x_chunks = []
        for g in range(KT // XC):
            xg = xp.tile([128, XC, T], BF16, tag="x", name=f"x{g}")
            nc.sync.dma_start(out=xg[:], in_=xT[:, g * XC:(g + 1) * XC, :])
            x_chunks.append(xg)

        def xk(k):
            return x_chunks[k // XC][:, k % XC, :]

        # ---- W prefetch (sync queue, FIFO after x so x has HBM priority) ----
        w_tiles = {}

        def load_w(ot):
            wq = wp.tile([128, KT, 128], BF16, tag="w", name=f"w{ot}")
            nc.sync.dma_start(out=wq[:], in_=wT[:, ot, :, :])
            w_tiles[ot] = wq

        load_w(0)
        load_w(1)
        bT_sb = cst.tile([ER, TO], BF16)
        nc.sync.dma_start(out=bT_sb[:], in_=bT[:])
        load_w(2)
        load_w(3)

        # ---- phase A: low.T = A_all.T^T @ x.T ; gate.T = g^T @ x.T ----
        low_ps = psA.tile([ER, T], F32, tag="low")
        gate_ps = psG.tile([E, T], F32, tag="g", name="gate")
        for k in range(KT):
            nc.tensor.matmul(low_ps[:], lhsT=a_sb[:, k, :], rhs=xk(k),
                             start=(k == 0), stop=(k == KT - 1))
            nc.tensor.matmul(gate_ps[:], lhsT=g_sb[:, k, :], rhs=xk(k),
                             start=(k == 0), stop=(k == KT - 1))

        # ---- gating math in [E, t] layout (vector/scalar/gpsimd queues;
        #      runs concurrently with phase B's first k-loops below) ----
        g_sbf = gw.tile([E, T], F32, tag="gsb")
        nc.vector.tensor_copy(g_sbf[:], gate_ps[:])
        m1b = gw.tile([E, T], F32, tag="m1b")
        nc.gpsimd.partition_all_reduce(m1b[:], g_sbf[:], channels=E,
                                       reduce_op=bass_isa.ReduceOp.max)
        eq = gw.tile([E, T], F32, tag="tmp", bufs=3, name="eq")
        nc.vector.tensor_tensor(eq[:], g_sbf[:], m1b[:], op=OP.is_equal)
        gm = gw.tile([E, T], F32, tag="tmp", bufs=3, name="gm")
        nc.vector.scalar_tensor_tensor(gm[:], in0=eq[:], scalar=-1e30, in1=g_sbf[:],
                                       op0=OP.mult, op1=OP.add)
        m2b = gw.tile([E, T], F32, tag="m2b")
        nc.gpsimd.partition_all_reduce(m2b[:], gm[:], channels=E,
                                       reduce_op=bass_isa.ReduceOp.max)
        diff = gw.tile([E, T], F32, tag="tmp", bufs=3, name="diff")
        nc.vector.tensor_sub(diff[:], g_sbf[:], m1b[:])
        ex = gw.tile([E, T], F32, tag="ex")
        nc.scalar.activation(ex[:], diff[:], mybir.ActivationFunctionType.Exp)
        mask = gw.tile([E, T], F32, tag="tmp", bufs=3, name="mask")
        nc.vector.tensor_tensor(mask[:], g_sbf[:], m2b[:], op=OP.is_ge)
        wn = gw.tile([E, T], F32, tag="wn")
        nc.vector.tensor_mul(wn[:], ex[:], mask[:])
        # denominator 1 + exp(m2 - m1), broadcast on all 8 rows
        dmb = gw.tile([E, T], F32, tag="tmp", bufs=3, name="dmb")
        nc.vector.tensor_sub(dmb[:], m2b[:], m1b[:])
        edb = gw.tile([E, T], F32, tag="edb")
        nc.scalar.activation(edb[:], dmb[:], mybir.ActivationFunctionType.Exp)
        denb = gw.tile([E, T], F32, tag="tmp", bufs=3, name="denb")
        nc.vector.tensor_scalar_add(denb[:], edb[:], 1.0)
        recb = gw.tile([E, T], F32, tag="recb")
        nc.vector.reciprocal(recb[:], denb[:])
        wsc = gw.tile([E, T], BF16, tag="wsc")
        nc.vector.scalar_tensor_tensor(wsc[:], in0=wn[:], scalar=SCALING, in1=recb[:],
                                       op0=OP.mult, op1=OP.mult)

        # ---- phase B: out.T tile = W-tile^T @ x.T (+ B-tile^T @ low_w.T) ----
        def kloop(ot):
            pb = psB.tile([128, T], F32, tag="pb", name=f"pb{ot}")
            for k in range(KT):
                nc.tensor.matmul(pb[:], lhsT=w_tiles[ot][:, k, :], rhs=xk(k),
                                 start=(k == 0), stop=False)
            return pb

        pbs = {0: kloop(0)}
        load_w(4)
        pbs[1] = kloop(1)
        load_w(5)

        # wrep: replicate each expert weight over its 16 ranks via tiny matmul.
        # Issued on the tensor queue after two k-loops (~14us) so the gating
        # chain above has already finished by the time the PE reaches it.
        wrep_ps = psG.tile([ER, T], F32, tag="g", name="wrep")
        nc.tensor.matmul(wrep_ps[:], lhsT=Rm_sb[:], rhs=wsc[:],
                         start=True, stop=True)
        # low_w.T = low.T * w_rep  (copy wrep to SBUF first: DVE has a single
        # PSUM read port, two-PSUM-operand tensor_tensor is illegal)
        wrep_sb = gw.tile([ER, T], F32, tag="wrepsb")
        nc.scalar.copy(wrep_sb[:], wrep_ps[:])
        lowT_sb = gw.tile([ER, T], BF16, tag="lowT")
        nc.vector.tensor_tensor(lowT_sb[:], low_ps[:], wrep_sb[:], op=OP.mult)

        for ot in range(OTN):
            nxt = ot + 2
            if nxt < OTN:
                pbs[nxt] = kloop(nxt)
                if nxt + 4 < OTN:
                    load_w(nxt + 4)
            pb = pbs.pop(ot)
            nc.tensor.matmul(pb[:], lhsT=bT_sb[:, ot * 128:(ot + 1) * 128],
                             rhs=lowT_sb[:], start=False, stop=True)
            o_sb = outp.tile([128, T], F32, tag="o", name=f"o{ot}")
            nc.vector.tensor_scalar(o_sb[:], pb[:],
                                    scalar1=bias_sb[:, ot:ot + 1], scalar2=None,
                                    op0=OP.add)
            nc.gpsimd.dma_start(out=out[:, ot, :], in_=o_sb[:])


def build_module(debug=False):
    nc = bacc.Bacc("TRN2", target_bir_lowering=False, debug=debug)
    xT = nc.dram_tensor("xT", [128, KT, T], BF16, kind="ExternalInput")
    wT = nc.dram_tensor("wT", [128, OTN, KT, 128], BF16, kind="ExternalInput")
    aT = nc.dram_tensor("aT", [128, KT, ER], BF16, kind="ExternalInput")
    gT = nc.dram_tensor("gT", [128, KT, E], BF16, kind="ExternalInput")
    bT = nc.dram_tensor("bT", [ER, TO], BF16, kind="ExternalInput")
    bias2 = nc.dram_tensor("bias2", [128, OTN], F32, kind="ExternalInput")
    Rm = nc.dram_tensor("Rm", [E, ER], BF16, kind="ExternalInput")
    out = nc.dram_tensor("out", [128, OTN, T], F32, kind="ExternalOutput")
    with tile.TileContext(nc) as tc:
        build_body(nc, tc, (xT, wT, aT, gT, bT, bias2, Rm, out))
    nc.compile()
    return nc


def shard_inputs(x, gate_w, base_w, base_b, lora_A, lora_B):
    """FULL inputs -> list of 8 per-core input maps (host-side, free)."""
    x = np.asarray(x, dtype=np.float32)
    gate_w = np.asarray(gate_w, dtype=np.float32)
    base_w = np.asarray(base_w, dtype=np.float32)
    base_b = np.asarray(base_b, dtype=np.float32)
    lora_A = np.asarray(lora_A, dtype=np.float32)
    lora_B = np.asarray(lora_B, dtype=np.float32)

    xf = x.reshape(B * S, D)
    # replicated smalls
    gT = np.ascontiguousarray(
        gate_w.T.reshape(KT, 128, E).transpose(1, 0, 2)).astype(NP_BF16)
    A_flat = lora_A.reshape(ER, D)
    aT = np.ascontiguousarray(
        A_flat.T.reshape(KT, 128, ER).transpose(1, 0, 2)).astype(NP_BF16)
    B_flat = lora_B.transpose(0, 2, 1).reshape(ER, O)   # [er, o]
    bT = np.ascontiguousarray(B_flat).astype(NP_BF16)
    Rm = np.repeat(np.eye(E, dtype=np.float32), R, axis=1).astype(NP_BF16)
    # replicated full base weight, bf16, contraction-on-partitions layout
    wT = np.ascontiguousarray(
        base_w.reshape(OTN, 128, KT, 128).transpose(3, 0, 2, 1)).astype(NP_BF16)
    bias2 = np.ascontiguousarray(base_b.reshape(OTN, 128).T)

    in_maps = []
    for c in range(N_CORES):
        x_c = xf[c * T:(c + 1) * T]                     # [T, D]
        xT = np.ascontiguousarray(
            x_c.T.reshape(KT, 128, T).transpose(1, 0, 2)).astype(NP_BF16)
        in_maps.append({"xT": xT, "wT": wT, "aT": aT, "gT": gT,
                        "bT": bT, "bias2": bias2, "Rm": Rm})
    return in_maps


def gather_outputs(results):
    """list of 8 per-core result maps -> FULL output [B, S, O]."""
    full = np.empty((B * S, O), dtype=np.float32)
    for c in range(N_CORES):
        oc = results[c]["out"]                          # [128, OTN, T]
        full[c * T:(c + 1) * T, :] = oc.transpose(2, 1, 0).reshape(T, O)
    return full.reshape(B, S, O)


_NC_CACHE = {}


def _get_module():
    if "nc" not in _NC_CACHE:
        _NC_CACHE["nc"] = build_module()
    return _NC_CACHE["nc"]


def run_sharded(in_maps, **run_kwargs):
    nc = _get_module()
    return run_bass_kernel_spmd(nc, in_maps, list(range(N_CORES)), **run_kwargs)


def kernel(x, gate_w, base_w, base_b, lora_A, lora_B):
    in_maps = shard_inputs(x, gate_w, base_w, base_b, lora_A, lora_B)
    res = run_sharded(in_maps)
    return gather_outputs(res.results)
